# revision 12
# baseline (speedup 1.0000x reference)
"""LSEP loss kernel for Trainium2 (8 NeuronCores, SPMD data-parallel).

loss = log1p( sum_i [ (sum_{c: t=0} exp(x_ic)) * (sum_{c: t=1} exp(-x_ic)) ] )

Batch (32768) sharded across 8 cores (4096 rows each); per core the shard is
viewed as [128 partitions, 32 samples x 1000 classes] and streamed in
column-chunks. Two alternating per-chunk schemes split the per-element work
between ScalarE (exp) and VectorE so both stay under the DMA roofline:

Scheme A (ACT-heavy, 1 DVE pass + 2 ACT passes):
  a = x - BIG*t
  s_neg = sum exp(a)        (ACT accum; t==1 entries underflow to 0)
  s_pos = sum exp(-a - BIG) (ACT accum; t==0 entries underflow to 0)

Scheme B (DVE-heavy, 3 DVE passes + 1 ACT pass):
  m = 1 - 2t (+-1), a = m*x, e = exp(a)
  S = sum e   (ACT accum)     D = sum e*m  (DVE tensor_tensor_reduce accum)
  s_neg*s_pos = (S^2 - D^2)/4

Epilogue reduces both to one [128,2] partial per core; host sums + log1p.
"""

import numpy as np

BATCH = 32768
C = 1000
N_CORES = 8
ROWS = BATCH // N_CORES          # 4096 rows per core
P = 128                          # SBUF partitions
SPR = ROWS // P                  # 32 samples per partition
BIG = 50.0
CHUNKS = [1, 1] + [2] * 15       # sample-columns per chunk; sum == 32

_CACHE = {}


def _build_nc():
    import concourse.bacc as bacc
    import concourse.mybir as mybir
    from concourse.tile import TileContext

    f32 = mybir.dt.float32
    i32 = mybir.dt.int32
    Exp = mybir.ActivationFunctionType.Exp
    Alu = mybir.AluOpType

    assert sum(CHUNKS) == SPR
    wmax = max(CHUNKS) * C
    n_a_cols = sum(w for i, w in enumerate(CHUNKS) if i % 2 == 0)
    n_b_cols = SPR - n_a_cols

    nc = bacc.Bacc()
    x = nc.declare_dram_parameter("input", [ROWS, C], f32, isOutput=False)
    t = nc.declare_dram_parameter("target", [ROWS, C], i32, isOutput=False)
    out = nc.declare_dram_parameter("partial", [P, 2], f32, isOutput=True)

    # partition p holds samples [p*32, (p+1)*32), 32000 contiguous floats
    xv = x.rearrange("(p s) c -> p (s c)", p=P)
    tv = t.rearrange("(p s) c -> p (s c)", p=P)

    with TileContext(nc) as tc:
        with (
            tc.tile_pool(name="io", bufs=4) as io,
            tc.tile_pool(name="eb", bufs=3) as ebp,
            tc.tile_pool(name="acc", bufs=1) as accp,
            tc.tile_pool(name="ps", bufs=1, space="PSUM") as psp,
        ):
            sn = psp.tile([P, n_a_cols], f32)   # scheme A: sum exp(a)
            sp = psp.tile([P, n_a_cols], f32)   # scheme A: sum exp(-a-BIG)
            S = psp.tile([P, n_b_cols], f32)    # scheme B: sum e
            D = accp.tile([P, n_b_cols], f32)   # scheme B: sum e*m (DVE accum)
            escr = psp.tile([P, C], f32)        # ACT main-out scratch (discarded)
            dscr = accp.tile([P, C], f32)       # DVE ttr main-out scratch
            bneg = accp.tile([P, 1], f32)       # bias AP holding -BIG
            nc.vector.memset(bneg[:], -BIG)
            off = 0
            ka = 0
            kb = 0
            for ci, ncols in enumerate(CHUNKS):
                w = ncols * C
                xt = io.tile([P, wmax], f32, tag="x")
                tt = io.tile([P, wmax], i32, tag="t")
                at = io.tile([P, wmax], f32, tag="a")
                nc.sync.dma_start(xt[:, :w], xv[:, off * C : off * C + w])
                nc.sync.dma_start(tt[:, :w], tv[:, off * C : off * C + w])
                if ci % 2 == 0:
                    # ---- scheme A ----
                    nc.vector.scalar_tensor_tensor(
                        at[:, :w], tt[:, :w], -BIG, xt[:, :w],
                        op0=Alu.mult, op1=Alu.add,
                    )
                    for j in range(ncols):
                        seg = at[:, j * C : (j + 1) * C]
                        nc.scalar.activation(
                            escr[:], seg, Exp, accum_out=sn[:, ka : ka + 1]
                        )
                        nc.scalar.activation(
                            escr[:], seg, Exp, scale=-1.0, bias=bneg[:],
                            accum_out=sp[:, ka : ka + 1],
                        )
                        ka += 1
                else:
                    # ---- scheme B ----
                    mt = io.tile([P, wmax], f32, tag="m")
                    et = ebp.tile([P, wmax], f32, tag="e")
                    # m = t*(-2) + 1  (single-src op, 2x DVE mode)
                    nc.vector.tensor_scalar(
                        mt[:, :w], tt[:, :w], -2.0, 1.0,
                        op0=Alu.mult, op1=Alu.add,
                    )
                    nc.vector.tensor_tensor(
                        at[:, :w], mt[:, :w], xt[:, :w], Alu.mult
                    )
                    for j in range(ncols):
                        seg = at[:, j * C : (j + 1) * C]
                        nc.scalar.activation(
                            et[:, j * C : (j + 1) * C], seg, Exp,
                            accum_out=S[:, kb : kb + 1],
                        )
                        # D accum: out = (e * 1.0) * m, accum_out = row-sum
                        # (tensor_tensor_reduce dies on HW; STT+accum works)
                        nc.vector.scalar_tensor_tensor(
                            dscr[:],
                            et[:, j * C : (j + 1) * C],
                            1.0,
                            mt[:, j * C : (j + 1) * C],
                            op0=Alu.mult, op1=Alu.mult,
                            accum_out=D[:, kb : kb + 1],
                        )
                        kb += 1
                off += ncols
            # epilogue:
            #   out[:,0] = sum_ka sn*sp          (scheme A product sums)
            #   out[:,1] = sum_kb (S^2 - D^2)    (scheme B; host divides by 4)
            snc = accp.tile([P, n_a_cols], f32)
            pa = accp.tile([P, n_a_cols], f32)
            sc = accp.tile([P, n_b_cols], f32)
            s2 = accp.tile([P, n_b_cols], f32)
            d2 = accp.tile([P, n_b_cols], f32)
            tot = accp.tile([P, 2], f32)
            nc.vector.tensor_copy(snc[:], sn[:])
            nc.vector.tensor_tensor(pa[:], snc[:], sp[:], Alu.mult)
            nc.vector.reduce_sum(tot[:, 0:1], pa[:], axis=mybir.AxisListType.X)
            nc.vector.tensor_copy(sc[:], S[:])
            nc.vector.tensor_tensor(s2[:], sc[:], sc[:], Alu.mult)
            nc.vector.tensor_tensor(d2[:], D[:], D[:], Alu.mult)
            nc.vector.tensor_tensor(s2[:], s2[:], d2[:], Alu.subtract)
            nc.vector.reduce_sum(tot[:, 1:2], s2[:], axis=mybir.AxisListType.X)
            nc.sync.dma_start(out[:], tot[:])
    # Bacc.compile() legalizes sync waits (ISA allows 1 wait/instruction;
    # extra waits become standalone EventSemaphore instructions).
    nc.compile()
    return nc


def _get_nc():
    if "nc" not in _CACHE:
        _CACHE["nc"] = _build_nc()
    return _CACHE["nc"]


def kernel(input, target):
    from concourse.bass_utils import run_bass_kernel_spmd

    x = np.ascontiguousarray(np.asarray(input, dtype=np.float32))
    t = np.ascontiguousarray(np.asarray(target, dtype=np.int32))
    assert x.shape == (BATCH, C) and t.shape == (BATCH, C)

    nc = _get_nc()
    in_maps = [
        {
            "input": x[i * ROWS : (i + 1) * ROWS],
            "target": t[i * ROWS : (i + 1) * ROWS],
        }
        for i in range(N_CORES)
    ]
    res = run_bass_kernel_spmd(nc, in_maps, list(range(N_CORES)))
    total = 0.0
    for r in res.results:
        p = r["partial"].astype(np.float64)
        total += float(p[:, 0].sum() + p[:, 1].sum() / 4.0)
    return np.asarray([np.log1p(total)], dtype=np.float32)


# revision 15
# speedup vs baseline: 1.0880x; 1.0880x over previous
"""LSEP loss kernel for Trainium2 (8 NeuronCores, SPMD data-parallel).

loss = log1p( sum_i [ (sum_{c: t=0} exp(x_ic)) * (sum_{c: t=1} exp(-x_ic)) ] )

Strategy: shard the batch (32768) across 8 cores (4096 rows each).
Per core, view the shard as [128 partitions, 32 samples x 1000 classes] and
stream column-chunks:
  a = x - BIG*t           (one DVE scalar_tensor_tensor op, int32 t cast on read)
  s_neg_row = sum exp(a)          -> exact exp(x) where t==0, ~0 where t==1
  s_pos_row = sum exp(-a - BIG)   -> exp(-x) where t==1, ~0 where t==0
(both exps via ScalarE activation free affine + accum_out row reduction into
PSUM accumulators). Epilogue: prod = s_neg*s_pos per sample, reduce,
DMA [128,1] partial per core; final scalar sum + log1p on host.

Chunk schedule [1,1,2,2,...]: small first chunks cut the pipeline ramp-in
(first EXP can start after 0.5 MB x2 instead of 2 MB x2).
"""

import numpy as np

BATCH = 32768
C = 1000
N_CORES = 8
ROWS = BATCH // N_CORES          # 4096 rows per core
P = 128                          # SBUF partitions
SPR = ROWS // P                  # 32 samples per partition
NSLC = SPR                       # accumulated sample-columns per partition
BIG = 50.0
# small chunks at both ends: fast pipeline ramp-in AND a short tail
# dependency chain after the last DMA completes
CHUNKS = [1, 1, 1] + [2] * 13 + [1, 1, 1]  # sum == 32

_CACHE = {}


def _build_nc():
    import concourse.bacc as bacc
    import concourse.mybir as mybir
    from concourse.tile import TileContext

    f32 = mybir.dt.float32
    i32 = mybir.dt.int32
    Exp = mybir.ActivationFunctionType.Exp
    Alu = mybir.AluOpType

    assert sum(CHUNKS) == NSLC
    wmax = max(CHUNKS) * C

    nc = bacc.Bacc()
    x = nc.declare_dram_parameter("input", [ROWS, C], f32, isOutput=False)
    t = nc.declare_dram_parameter("target", [ROWS, C], i32, isOutput=False)
    out = nc.declare_dram_parameter("partial", [P, 1], f32, isOutput=True)

    # partition p holds samples [p*32, (p+1)*32), 32000 contiguous floats
    xv = x.rearrange("(p s) c -> p (s c)", p=P)
    tv = t.rearrange("(p s) c -> p (s c)", p=P)

    with TileContext(nc) as tc:
        with (
            tc.tile_pool(name="io", bufs=4) as io,
            tc.tile_pool(name="acc", bufs=1) as accp,
            tc.tile_pool(name="ps", bufs=1, space="PSUM") as psp,
        ):
            sn = psp.tile([P, NSLC], f32)
            sp = psp.tile([P, NSLC], f32)
            escr = psp.tile([P, C], f32)  # ACT main output scratch (discarded)
            bneg = accp.tile([P, 1], f32)  # bias AP holding -BIG
            nc.vector.memset(bneg[:], -BIG)
            off = 0
            for ncols in CHUNKS:
                w = ncols * C
                xt = io.tile([P, wmax], f32, tag="x")
                tt = io.tile([P, wmax], i32, tag="t")
                at = io.tile([P, wmax], f32, tag="a")
                nc.sync.dma_start(xt[:, :w], xv[:, off * C : off * C + w])
                nc.sync.dma_start(tt[:, :w], tv[:, off * C : off * C + w])
                # a = (t * -BIG) + x
                nc.vector.scalar_tensor_tensor(
                    at[:, :w], tt[:, :w], -BIG, xt[:, :w],
                    op0=Alu.mult, op1=Alu.add,
                )
                for j in range(ncols):
                    k = off + j
                    seg = at[:, j * C : (j + 1) * C]
                    # s_neg: exp(a); masked (t==1) entries exp(x-50) ~ 0
                    nc.scalar.activation(
                        escr[:], seg, Exp, accum_out=sn[:, k : k + 1]
                    )
                    # s_pos: exp(-a-50); masked (t==0) entries exp(-x-50) ~ 0
                    nc.scalar.activation(
                        escr[:], seg, Exp, scale=-1.0, bias=bneg[:],
                        accum_out=sp[:, k : k + 1],
                    )
                off += ncols
            # epilogue: prod per sample-column, reduce, write [128,1] partial
            sns = accp.tile([P, NSLC], f32)
            prod = accp.tile([P, NSLC], f32)
            tot = accp.tile([P, 1], f32)
            nc.vector.tensor_copy(sns[:], sn[:])
            nc.vector.tensor_tensor(prod[:], sns[:], sp[:], Alu.mult)
            nc.vector.reduce_sum(tot[:], prod[:], axis=mybir.AxisListType.X)
            # out-DMA on the ACT HWDGE ring: the sync ring's FIFO still
            # holds input-DMA completions at this point
            nc.scalar.dma_start(out[:], tot[:])
    # Bacc.compile() legalizes sync waits (ISA allows 1 wait/instruction;
    # extra waits become standalone EventSemaphore instructions).
    nc.compile()
    return nc


def _get_nc():
    if "nc" not in _CACHE:
        _CACHE["nc"] = _build_nc()
    return _CACHE["nc"]


def kernel(input, target):
    from concourse.bass_utils import run_bass_kernel_spmd

    x = np.ascontiguousarray(np.asarray(input, dtype=np.float32))
    t = np.ascontiguousarray(np.asarray(target, dtype=np.int32))
    assert x.shape == (BATCH, C) and t.shape == (BATCH, C)

    nc = _get_nc()
    in_maps = [
        {
            "input": x[i * ROWS : (i + 1) * ROWS],
            "target": t[i * ROWS : (i + 1) * ROWS],
        }
        for i in range(N_CORES)
    ]
    res = run_bass_kernel_spmd(nc, in_maps, list(range(N_CORES)))
    total = 0.0
    for r in res.results:
        total += float(np.sum(r["partial"].astype(np.float64)))
    return np.asarray([np.log1p(total)], dtype=np.float32)
